# revision 11
# baseline (speedup 1.0000x reference)
"""Dual (real/phase) multi-head attention for TRN2, 8 NeuronCores.

Sharding: core c handles batch b = c//2 and query-row half = c%2 (1024 rows),
all 8 heads, full key length. No collectives; fc contracts fully per core.

Per-core pipeline:
  P1: consts + weight transposes (shared slot); x^T via PE transpose;
      projections: qT/kT per-head stacks [128=(2*dk), L] fp32r (w_q and b_q
      pre-scaled by 1/TEMP on host), v natural stacks [lk, 128=(vr|vp)] bf16.
  P2: per (head, qblock): scores = qstack^T kstack - 240*mask (fp8 mask MM in
      the same PSUM group); exp+rowsum fused on ScalarE (half tiles); in-place
      normalize on VectorE; DMA attn out; bf16 cast; SBUF->SBUF DMA transpose;
      attn@v as v_stack^T @ attn^T on PE per qblock-pair; o^T staged to DRAM.
  P3: fc (bf16) + residual + layernorm.
"""
import numpy as np

import concourse.bass as bass
import concourse.mybir as mybir
import concourse.tile as tile
from concourse import bacc
from concourse.bass_utils import run_bass_kernel_spmd
from concourse.masks import make_identity

B, L, D, H, DK, DV = 4, 2048, 512, 8, 64, 64
LH = L // 2
TEMP = float(np.sqrt(2.0 * DK))
EPS = 1e-5
MASKC = -240.0  # exp(s - 240) == 0.0 in fp32 for |s| <= ~100

f32 = mybir.dt.float32
f32r = mybir.dt.float32r
bf16 = mybir.dt.bfloat16
f8e4 = mybir.dt.float8e4
AF = mybir.ActivationFunctionType
ALU = mybir.AluOpType

_CACHED_NC = None


def _build_weight_T(nc, pool, psum, w_d, iden, out_dtype, wt_pool=None,
                    wt_tag="wT"):
    """w [HD, D] natural in DRAM -> wT sbuf tile [128, 4, 512]:
    wT[p, j, n] = w[n, j*128+p]."""
    tp = wt_pool if wt_pool is not None else pool
    wT = tp.tile([128, 4, 512], out_dtype, name=wt_tag, tag=wt_tag, bufs=1)
    for c in range(4):
        wnat = pool.tile([128, 512], f32, name="wnat", tag="wnat", bufs=2)
        nc.sync.dma_start(wnat[:], w_d[c * 128:(c + 1) * 128, :])
        for j in range(4):
            ps = psum.tile([128, 128], f32, name="wtp", tag="wtp", bufs=2)
            nc.tensor.transpose(ps[:], wnat[:, j * 128:(j + 1) * 128], iden[:])
            nc.scalar.copy(wT[:, j, c * 128:(c + 1) * 128], ps[:])
    return wT


def _build_xT(nc, xT, pool, psum, x_d, nrows, iden):
    """Fill xT [128, 4, nrows] (fp32r) with x^T: xT[p, j, l] = x[l, j*128+p]."""
    for sb4 in range(nrows // 512):
        xn = pool.tile([128, 4, 512], f32, name="xn", tag="xn", bufs=2)
        for i in range(4):
            r = sb4 * 512 + i * 128
            nc.sync.dma_start(xn[:, i, :], x_d[r:r + 128, :])
        for j in range(4):
            ps = psum.tile([128, 512], f32, name="xtp", tag="xtp", bufs=2)
            for i in range(4):
                nc.tensor.transpose(
                    ps[:, i * 128:(i + 1) * 128],
                    xn[:, i, j * 128:(j + 1) * 128], iden[:])
            nc.scalar.copy(xT[:, j, sb4 * 512:(sb4 + 1) * 512], ps[:])


def build(stage=3):
    nc = bacc.Bacc("TRN2", target_bir_lowering=False)

    xq_r_d = nc.dram_tensor("xq_r", [LH, D], f32, kind="ExternalInput")
    xq_p_d = nc.dram_tensor("xq_p", [LH, D], f32, kind="ExternalInput")
    xk_r_d = nc.dram_tensor("xk_r", [L, D], f32, kind="ExternalInput")
    xk_p_d = nc.dram_tensor("xk_p", [L, D], f32, kind="ExternalInput")
    xv_r_d = nc.dram_tensor("xv_r", [L, D], f32, kind="ExternalInput")
    xv_p_d = nc.dram_tensor("xv_p", [L, D], f32, kind="ExternalInput")
    mask_d = nc.dram_tensor("mask8", [LH, L], f8e4, kind="ExternalInput")
    wq_d = nc.dram_tensor("w_q", [D, D], f32, kind="ExternalInput")
    wk_d = nc.dram_tensor("w_k", [D, D], f32, kind="ExternalInput")
    wv_d = nc.dram_tensor("w_v", [D, D], f32, kind="ExternalInput")
    wfc_d = nc.dram_tensor("w_fc", [D, D], f32, kind="ExternalInput")
    bq_d = nc.dram_tensor("b_q", [D], f32, kind="ExternalInput")
    bk_d = nc.dram_tensor("b_k", [D], f32, kind="ExternalInput")
    bv_d = nc.dram_tensor("b_v", [D], f32, kind="ExternalInput")
    bfc_d = nc.dram_tensor("b_fc", [D], f32, kind="ExternalInput")
    lng_d = nc.dram_tensor("ln_g", [D], f32, kind="ExternalInput")
    lnb_d = nc.dram_tensor("ln_b", [D], f32, kind="ExternalInput")

    out_r_d = nc.dram_tensor("out_r", [LH, D], f32, kind="ExternalOutput")
    out_p_d = nc.dram_tensor("out_p", [LH, D], f32, kind="ExternalOutput")
    attn_d = nc.dram_tensor("attn", [H, LH, L], f32, kind="ExternalOutput")

    # DRAM scratch for o^T between P2 and P3
    oTr_d = nc.dram_tensor("oTr_scr", [128, 4, 1024], bf16, kind="Internal")
    oTp_d = nc.dram_tensor("oTp_scr", [128, 4, 1024], bf16, kind="Internal")

    with tile.TileContext(nc) as tc:
        with (
            tc.tile_pool(name="persist", bufs=1) as pers,
            tc.tile_pool(name="small", bufs=2) as small,
        ):
            # ---------- consts ----------
            iden = pers.tile([128, 128], f32)
            make_identity(nc, iden[:])
            negi8 = pers.tile([128, 128], f8e4)
            make_identity(nc, negi8[:])
            nc.vector.tensor_scalar_mul(negi8[:], negi8[:], MASKC)

            onesf = small.tile([1, 512], f32, bufs=1)
            nc.vector.memset(onesf[:], 1.0)
            ones512_r = pers.tile([1, 512], f32r)
            nc.vector.tensor_copy(ones512_r[:], onesf[:])
            onescol_f = small.tile([1, 128], f32, bufs=1)
            nc.vector.memset(onescol_f[:], 1.0)
            onescol_r = pers.tile([1, 128], f32r)
            nc.vector.tensor_copy(onescol_r[:], onescol_f[:])
            onescol_bf = pers.tile([1, 128], bf16)
            nc.vector.memset(onescol_bf[:], 1.0)
            epsb = pers.tile([128, 1], f32)
            nc.vector.memset(epsb[:], EPS)

            def _row_r(d_t, name):
                raw = small.tile([1, 512], f32, name=f"{name}_raw",
                                 tag="rowraw", bufs=1)
                nc.sync.dma_start(raw[:], d_t[None, :])
                row = pers.tile([1, 512], f32r, name=name, tag=name, bufs=1)
                nc.vector.tensor_copy(row[:], raw[:])
                return row

            bq_row = _row_r(bq_d, "bq_row")
            bk_row = _row_r(bk_d, "bk_row")
            bv_row = _row_r(bv_d, "bv_row")
            bfc_raw = small.tile([1, 512], f32, bufs=1)
            nc.sync.dma_start(bfc_raw[:], bfc_d[None, :])
            bfc_row_bf = pers.tile([1, 512], bf16)
            nc.vector.tensor_copy(bfc_row_bf[:], bfc_raw[:])

            # ln g/b broadcast tiles [128, 512] via K=1 fp32 matmul
            with tc.tile_pool(name="p0ps", bufs=2, space="PSUM") as p0ps:
                g_bcast = pers.tile([128, 512], f32)
                b_bcast = pers.tile([128, 512], f32)
                for vec_d, dst in ((lng_d, g_bcast), (lnb_d, b_bcast)):
                    vrow = small.tile([1, 512], f32, name="vrow", tag="vrow",
                                      bufs=1)
                    nc.sync.dma_start(vrow[:], vec_d[None, :])
                    ps = p0ps.tile([128, 512], f32, name="bc_ps", tag="bc")
                    nc.tensor.matmul(ps[:], onescol_f[:], vrow[:],
                                     start=True, stop=True)
                    nc.scalar.copy(dst[:], ps[:])

            # persistent across P1/P2
            qT = pers.tile([128, 8, 1024], f32r)   # [0:64,h]=qr^T [64:,h]=qp^T
            kT = pers.tile([128, 8, 2048], f32r)
            v_st = pers.tile([128, 16, 8, 128], bf16)  # [lk%128,lk//128,h,vr|vp]

            # ---------- P1: weights^T, x^T, projections ----------
            with (
                tc.tile_pool(name="p1sb", bufs=1) as p1sb,
                tc.tile_pool(name="p1ps", bufs=2, space="PSUM") as p1ps,
            ):
                # wfcT (persistent, bf16)
                wfcT = _build_weight_T(nc, p1sb, p1ps, wfc_d, iden, bf16,
                                       wt_pool=pers, wt_tag="wfcT")

                # --- v projections ---
                wvT = _build_weight_T(nc, p1sb, p1ps, wv_d, iden, f32r)
                for ti, xv_d in ((0, xv_r_d), (1, xv_p_d)):
                    for half in range(2):
                        xT = p1sb.tile([128, 4, 1024], f32r, name="xT",
                                       tag="xT", bufs=1)
                        _build_xT(nc, xT, p1sb, p1ps,
                                  xv_d[half * 1024:(half + 1) * 1024, :],
                                  1024, iden)
                        for lc8 in range(8):
                            lc = half * 8 + lc8
                            ps = p1ps.tile([128, 512], f32, name="vps",
                                           tag="projps", bufs=2)
                            for k in range(4):
                                nc.tensor.matmul(
                                    ps[:], xT[:, k, lc8 * 128:(lc8 + 1) * 128],
                                    wvT[:, k, :], start=(k == 0), stop=False)
                            nc.tensor.matmul(ps[:], onescol_r[:], bv_row[:],
                                             start=False, stop=True)
                            nc.scalar.copy(
                                v_st[:, lc, :, ti * 64:(ti + 1) * 64],
                                ps[:].rearrange("p (h d) -> p h d", h=H))

                # --- q projections ---
                wqT = _build_weight_T(nc, p1sb, p1ps, wq_d, iden, f32r)
                for ti, xq_d in ((0, xq_r_d), (1, xq_p_d)):
                    xT = p1sb.tile([128, 4, 1024], f32r, name="xT", tag="xT",
                                   bufs=1)
                    _build_xT(nc, xT, p1sb, p1ps, xq_d, 1024, iden)
                    for m in range(4):
                        for nw in range(2):
                            ps = p1ps.tile([128, 512], f32, name="qps",
                                           tag="projps", bufs=2)
                            for k in range(4):
                                nc.tensor.matmul(
                                    ps[:], wqT[:, k, m * 128:(m + 1) * 128],
                                    xT[:, k, nw * 512:(nw + 1) * 512],
                                    start=(k == 0), stop=False)
                            nc.tensor.matmul(
                                ps[:], bq_row[0:1, m * 128:(m + 1) * 128],
                                ones512_r[:], start=False, stop=True)
                            col = nw * 512
                            for hh in range(2):
                                nc.scalar.copy(
                                    qT[ti * 64:(ti + 1) * 64, 2 * m + hh,
                                       col:col + 512],
                                    ps[hh * 64:(hh + 1) * 64, :])

                # --- k projections ---
                wkT = _build_weight_T(nc, p1sb, p1ps, wk_d, iden, f32r)
                for ti, xk_d in ((0, xk_r_d), (1, xk_p_d)):
                    for half in range(2):
                        xT = p1sb.tile([128, 4, 1024], f32r, name="xT",
                                       tag="xT", bufs=1)
                        _build_xT(nc, xT, p1sb, p1ps,
                                  xk_d[half * 1024:(half + 1) * 1024, :],
                                  1024, iden)
                        for m in range(4):
                            for nw in range(2):
                                ps = p1ps.tile([128, 512], f32, name="kps",
                                               tag="projps", bufs=2)
                                for k in range(4):
                                    nc.tensor.matmul(
                                        ps[:], wkT[:, k, m * 128:(m + 1) * 128],
                                        xT[:, k, nw * 512:(nw + 1) * 512],
                                        start=(k == 0), stop=False)
                                nc.tensor.matmul(
                                    ps[:], bk_row[0:1, m * 128:(m + 1) * 128],
                                    ones512_r[:], start=False, stop=True)
                                col = half * 1024 + nw * 512
                                for hh in range(2):
                                    nc.scalar.copy(
                                        kT[ti * 64:(ti + 1) * 64, 2 * m + hh,
                                           col:col + 512],
                                        ps[hh * 64:(hh + 1) * 64, :])

            # ---------- P2: attention ----------
            with (
                tc.tile_pool(name="late", bufs=1) as late,
                tc.tile_pool(name="p2sb", bufs=2) as p2sb,
                tc.tile_pool(name="p2ps", bufs=2, space="PSUM") as p2ps,
            ):
                mask_all = late.tile([128, 8, 2048], f8e4)
                for qb in range(8):
                    nc.sync.dma_start(mask_all[:, qb, :],
                                      mask_d[qb * 128:(qb + 1) * 128, :])

                for h in range(H if stage >= 2 else 0):
                    Etr = p2sb.tile([128, 2, 16, 128], bf16, name="Etr",
                                    tag="Etr", bufs=2)
                    for qb in range(8):
                        S = p2ps.tile([128, 2048], f32, name="S", tag="S")
                        for w in range(4):
                            sl = slice(w * 512, (w + 1) * 512)
                            nc.tensor.matmul(
                                S[:, sl], qT[:, h, qb * 128:(qb + 1) * 128],
                                kT[:, h, sl], start=True, stop=False)
                            nc.tensor.matmul(S[:, sl], negi8[:],
                                             mask_all[:, qb, sl],
                                             start=False, stop=True)
                        rs = [None, None]
                        Eh = [None, None]
                        for hf in range(2):
                            Eh[hf] = p2sb.tile([128, 1024], f32, name="E",
                                               tag="E")
                            rs[hf] = small.tile([128, 1], f32, name="rs",
                                                tag="rs", bufs=4)
                            nc.scalar.activation(
                                Eh[hf][:], S[:, hf * 1024:(hf + 1) * 1024],
                                AF.Exp, accum_out=rs[hf][:])
                        rtot = small.tile([128, 1], f32, name="rtot",
                                          tag="rtot", bufs=4)
                        nc.vector.tensor_add(rtot[:], rs[0][:], rs[1][:])
                        rcp = small.tile([128, 1], f32, name="rcp", tag="rcp",
                                         bufs=4)
                        nc.vector.reciprocal(rcp[:], rtot[:])
                        for hf in range(2):
                            nc.vector.tensor_scalar_mul(Eh[hf][:], Eh[hf][:],
                                                        rcp[:])
                            nc.sync.dma_start(
                                attn_d[h, qb * 128:(qb + 1) * 128,
                                       hf * 1024:(hf + 1) * 1024], Eh[hf][:])
                            Ebf = p2sb.tile([128, 1024], bf16, name="Ebf",
                                            tag="Ebf")
                            nc.vector.tensor_copy(Ebf[:], Eh[hf][:])
                            nc.sync.dma_start_transpose(
                                Etr[:, qb % 2, hf * 8:(hf + 1) * 8, :],
                                Ebf[:])
                        if qb % 2 == 1:
                            pair = qb // 2
                            oPs = p2ps.tile([128, 2048], f32, name="oPs",
                                            tag="S")
                            for c in range(16):
                                nc.tensor.matmul(
                                    oPs[:, :256], v_st[:, c, h, :],
                                    Etr[:, :, c, :], start=(c == 0),
                                    stop=(c == 15))
                            for oT_d, hv in ((oTr_d, 0), (oTp_d, 1)):
                                stg = p2sb.tile([64, 256], bf16, name="stg",
                                                tag="stg", bufs=4)
                                nc.scalar.copy(stg[:],
                                               oPs[hv * 64:(hv + 1) * 64,
                                                   :256])
                                nc.sync.dma_start(
                                    oT_d[(h % 2) * 64:(h % 2 + 1) * 64,
                                         h // 2,
                                         pair * 256:(pair + 1) * 256],
                                    stg[:])

            # ---------- P3: fc + residual + layernorm ----------
            with (
                tc.tile_pool(name="p3sb", bufs=2) as p3sb,
                tc.tile_pool(name="p3ps", bufs=2, space="PSUM") as p3ps,
            ):
                for oT_d, res_d, out_d in (((oTr_d, xq_r_d, out_r_d),
                                            (oTp_d, xq_p_d, out_p_d))
                                           if stage >= 3 else ()):
                    oT = p3sb.tile([128, 4, 1024], bf16, name="oT", tag="oT",
                                   bufs=2)
                    nc.sync.dma_start(oT[:], oT_d[:])
                    for lc in range(8):
                        rsl = slice(lc * 128, (lc + 1) * 128)
                        ps = p3ps.tile([128, 512], f32, name="fc", tag="fc")
                        for j in range(4):
                            nc.tensor.matmul(ps[:], oT[:, j, rsl],
                                             wfcT[:, j, :],
                                             start=(j == 0), stop=False)
                        nc.tensor.matmul(ps[:], onescol_bf[:], bfc_row_bf[:],
                                         start=False, stop=True)
                        res = p3sb.tile([128, 512], f32, name="res", tag="res")
                        nc.sync.dma_start(res[:], res_d[rsl, :])
                        t = p3sb.tile([128, 512], f32, name="t", tag="t")
                        nc.vector.tensor_add(t[:], ps[:], res[:])
                        bns = small.tile([128, 6], f32, name="bns", tag="bns")
                        nc.vector.bn_stats(bns[:], t[:])
                        bna = small.tile([128, 2], f32, name="bna", tag="bna")
                        nc.vector.bn_aggr(bna[:], bns[:])
                        std = small.tile([128, 1], f32, name="std_t",
                                         tag="std")
                        nc.scalar.activation(std[:], bna[:, 1:2], AF.Sqrt,
                                             bias=epsb[:])
                        rsv = small.tile([128, 1], f32, name="rsv_t",
                                         tag="rsv")
                        nc.vector.reciprocal(rsv[:], std[:])
                        tcn = p3sb.tile([128, 512], f32, name="tcn", tag="tcn")
                        nc.vector.tensor_scalar_sub(tcn[:], t[:], bna[:, 0:1])
                        tn = p3sb.tile([128, 512], f32, name="tn", tag="tn")
                        nc.vector.tensor_scalar_mul(tn[:], tcn[:], rsv[:])
                        tg = p3sb.tile([128, 512], f32, name="tg", tag="tg")
                        nc.vector.tensor_mul(tg[:], tn[:], g_bcast[:])
                        outsb = p3sb.tile([128, 512], f32, name="outsb",
                                          tag="outsb")
                        nc.vector.tensor_add(outsb[:], tg[:], b_bcast[:])
                        nc.sync.dma_start(out_d[rsl, :], outsb[:])

    nc.finalize()
    return nc


def _get_nc():
    global _CACHED_NC
    if _CACHED_NC is None:
        import os
        _CACHED_NC = build(stage=int(os.environ.get("KERNEL_STAGE", "3")))
    return _CACHED_NC


def kernel(q_real, k_real, v_real, q_phase, k_phase, v_phase, mask,
           w_q, b_q, w_k, b_k, w_v, b_v, w_fc, b_fc, ln_g, ln_b):
    f8np = mybir.dt.np(f8e4)
    wq_s = (np.asarray(w_q, np.float32) / TEMP)
    bq_s = (np.asarray(b_q, np.float32) / TEMP)
    mask8 = np.asarray(mask).astype(np.float32).astype(f8np)

    common = {
        "w_q": wq_s, "w_k": np.asarray(w_k, np.float32),
        "w_v": np.asarray(w_v, np.float32),
        "w_fc": np.asarray(w_fc, np.float32),
        "b_q": bq_s, "b_k": np.asarray(b_k, np.float32),
        "b_v": np.asarray(b_v, np.float32),
        "b_fc": np.asarray(b_fc, np.float32),
        "ln_g": np.asarray(ln_g, np.float32),
        "ln_b": np.asarray(ln_b, np.float32),
    }
    in_maps = []
    for c in range(8):
        b, half = c // 2, c % 2
        rows = slice(half * LH, (half + 1) * LH)
        in_maps.append({
            "xq_r": np.ascontiguousarray(q_real[b, rows]),
            "xq_p": np.ascontiguousarray(q_phase[b, rows]),
            "xk_r": np.ascontiguousarray(k_real[b]),
            "xk_p": np.ascontiguousarray(k_phase[b]),
            "xv_r": np.ascontiguousarray(v_real[b]),
            "xv_p": np.ascontiguousarray(v_phase[b]),
            "mask8": np.ascontiguousarray(mask8[b, rows]),
            **common,
        })

    import os
    nc = _get_nc()
    br = run_bass_kernel_spmd(nc, in_maps, core_ids=list(range(8)),
                              trace=os.environ.get("KERNEL_TRACE") == "1")
    globals()["LAST_RESULTS"] = br
    results = br.results

    out_r = np.empty((B, L, D), np.float32)
    out_p = np.empty((B, L, D), np.float32)
    attn = np.empty((H * B, L, L), np.float32)
    for c in range(8):
        b, half = c // 2, c % 2
        rows = slice(half * LH, (half + 1) * LH)
        out_r[b, rows] = results[c]["out_r"]
        out_p[b, rows] = results[c]["out_p"]
        for h in range(H):
            attn[h * B + b, rows] = results[c]["attn"][h]
    return out_r, out_p, attn
